# revision 2
# baseline (speedup 1.0000x reference)
"""FastGTLayer GNN message passing on 8 Trainium2 NeuronCores — v2.

Strategy (destination-window sharded, fp16 gather + selection-matmul scatter):
- Host: softmax(weight) -> per-edge per-channel weights w_c = filt[c,t]*ev[t,e].
  Edges sharded by destination row range (6250 rows/core) and sorted by row.
  Each core's sorted edge list is segmented greedily into "windows": at most
  KT*128 edges AND spanning at most R=32 destination rows.  Every window gets
  exactly KT 128-edge tiles (tail padded with inert slots), so the device
  program is fully uniform — no per-block quota vector, ~0.5% padding
  (vs ~12% for per-block max-over-cores quotas).
- Device (SPMD, one program on 8 cores): for each chunk of CB windows,
  dma_gather fetches H rows (both channels interleaved, 256B/edge fp16) by
  int16 index with a biased base; DVE builds weighted one-hot selection
  matrices sel_c[e,r] = w_c[e]*(rowl[e]==r) in fp16; PE scatter-adds via
  fp16 matmul psum_c[64f, 32r] += g_c^T @ sel_c (4x the fp32 matmul rate).
  PSB consecutive windows accumulate into one [64, PSB*32] psum tile per
  channel; ACT evicts whole groups; HWDGE writes [128,(c,d)] x cols to HBM.
- Host: windows of core k map to row ranges [base(k,b), base(k,b)+32); the
  unshard scatter-adds window columns into out[C, N, D] (windows may split
  one row across two windows; np.add.at handles the overlap).
"""
import sys
if "/opt/trn_rl_repo" not in sys.path:
    sys.path.insert(0, "/opt/trn_rl_repo")

import numpy as np

C, T, N, E, D = 2, 4, 50000, 400000, 64
M = T * E
NCORES = 8
RPC = N // NCORES          # 6250 destination rows per core
R = 32                     # max rows per window (psum window)
KT = 7                     # tiles (x128 edges) per window
K = KT * 128               # max edges per window
BIAS = N - 32768           # 17232; idx = col - BIAS in [-17232, 32767]
PADCOL = 40000             # pad slots gather this row (positive idx), weight 0
CB = 14                    # windows per gather chunk / psum group
PADROW = 33.0              # pad rowl value: never equals iota 0..31

PROG_KW = dict(fold=True, gbufs=5, selbufs=2)

_prog_cache = {}


def _build_program(nb, nqueues=4, gbufs=3, selbufs=3, pbufs=4, sbufs=2,
                   skip_gather=False, skip_compute=False, tiny_gather=False,
                   fold=False, no_ratio=False, no_mm=False,
                   scratch=16384, repeat=1, cb=None):
    """Build the SPMD Bass program for `nb` windows of KT tiles each."""
    from concourse import bacc, mybir
    import concourse.tile as tile
    from concourse.bass import AP

    cb = cb or CB
    tt = nb * KT
    nc = bacc.Bacc("TRN2", num_swdge_queues=nqueues, dynamic_dma_scratch_size=scratch)
    hpre = nc.dram_tensor("hpre", [N, 2 * D], mybir.dt.float16, kind="ExternalInput")
    idx = nc.dram_tensor("idx", [128, tt * 8], mybir.dt.int16, kind="ExternalInput")
    rowl = nc.dram_tensor("rowl", [128, tt], mybir.dt.float16, kind="ExternalInput")
    w0 = nc.dram_tensor("w0", [128, tt], mybir.dt.float16, kind="ExternalInput")
    w1 = nc.dram_tensor("rt" if fold else "w1", [128, tt], mybir.dt.float16,
                        kind="ExternalInput")
    iota = nc.dram_tensor("iota", [128, R], mybir.dt.float16, kind="ExternalInput")
    out_local = nc.dram_tensor("out_local", [128, nb * R], mybir.dt.float32,
                               kind="ExternalOutput")

    nchunks = (nb + cb - 1) // cb

    with tile.TileContext(nc) as tc:
        with tc.tile_pool(name="meta", bufs=1) as mp, \
             tc.tile_pool(name="gp", bufs=gbufs) as gp, \
             tc.tile_pool(name="selp", bufs=selbufs) as selp, \
             tc.tile_pool(name="stp", bufs=sbufs) as stp, \
             tc.tile_pool(name="pp", bufs=pbufs, space="PSUM") as pp:
            idx_t = mp.tile([128, tt * 8], mybir.dt.int16)
            rowl_t = mp.tile([128, tt], mybir.dt.float16)
            w0_t = mp.tile([128, tt], mybir.dt.float16)
            w1_t = mp.tile([128, tt], mybir.dt.float16)
            iota_t = mp.tile([128, R], mybir.dt.float16)

            nc.gpsimd.dma_start(out=idx_t[:], in_=idx[:])
            nc.gpsimd.dma_start(out=rowl_t[:], in_=rowl[:])
            nc.gpsimd.dma_start(out=w0_t[:], in_=w0[:])
            nc.gpsimd.dma_start(out=w1_t[:], in_=w1[:])
            nc.gpsimd.dma_start(out=iota_t[:], in_=iota[:])

            iota_ap = iota_t[:]

            for rep in range(repeat):
              for c in range(nchunks):
                b0 = c * cb
                b1 = min(b0 + cb, nb)
                nbc = b1 - b0                # windows in this chunk
                ct = nbc * KT                # tiles in this chunk
                nidx = ct * 128
                tb0 = b0 * KT                # first global tile of chunk

                g_t = gp.tile([128, ct, 2 * D], mybir.dt.float16, tag="g")
                if not skip_gather:
                    g_nidx = 128 if tiny_gather else nidx
                    g_out = g_t[:, 0:1, :] if tiny_gather else g_t[:]
                    nc.gpsimd.dma_gather(
                        g_out,
                        hpre[BIAS:, :],
                        idx_t[:, tb0 * 8: tb0 * 8 + g_nidx // 16],
                        g_nidx,
                        g_nidx,
                        2 * D,
                        queue_num=(c % nqueues),
                        single_packet=False,
                    )

                if fold and not skip_compute and not skip_gather and not no_ratio:
                    # fold channel-1 weights into gathered data:
                    # g1' = g1 * (w1/w0); both channels then share sel0
                    nc.vector.tensor_tensor(
                        out=g_t[:, :, D:2 * D], in0=g_t[:, :, D:2 * D],
                        in1=w1_t[:, tb0:tb0 + ct].to_broadcast([128, ct, D]),
                        op=mybir.AluOpType.mult)

                if no_mm:
                    stage = stp.tile([128, nbc * R], mybir.dt.float32, tag="st")
                    nc.vector.memset(stage[:], 0.0)
                    nc.sync.dma_start(out=out_local[:, b0 * R: b1 * R],
                                      in_=stage[:])
                    continue

                if fold and not skip_compute:
                    # chunk-wide selection build: 2 DVE ops for all ct tiles
                    psf = pp.tile([128, nbc * R], mybir.dt.float32, space="PSUM",
                                  tag="pf")
                    iota_c = AP(iota_ap.tensor, iota_ap.offset,
                                [iota_ap.ap[0], [0, ct], iota_ap.ap[1]])
                    sel_eq = selp.tile([128, ct, R], mybir.dt.float16, tag="se")
                    sel0 = selp.tile([128, ct, R], mybir.dt.float16, tag="s0")
                    nc.vector.tensor_tensor(
                        out=sel_eq[:],
                        in0=rowl_t[:, tb0:tb0 + ct].to_broadcast([128, ct, R]),
                        in1=iota_c, op=mybir.AluOpType.is_equal)
                    nc.vector.tensor_tensor(
                        out=sel0[:], in0=sel_eq[:],
                        in1=w0_t[:, tb0:tb0 + ct].to_broadcast([128, ct, R]),
                        op=mybir.AluOpType.mult)
                    for b in range(b0, b1):
                        lt0 = b * KT - tb0
                        so = (b - b0) * R
                        for k in range(KT):
                            nc.tensor.matmul(out=psf[:, so:so + R],
                                             lhsT=g_t[:, lt0 + k, :],
                                             rhs=sel0[:, lt0 + k, :],
                                             start=(k == 0), stop=(k == KT - 1))
                elif not skip_compute:
                    ps0 = pp.tile([64, nbc * R], mybir.dt.float32, space="PSUM",
                                  tag="p0")
                    ps1 = pp.tile([64, nbc * R], mybir.dt.float32, space="PSUM",
                                  tag="p1")
                for b in [] if (skip_compute or fold) else range(b0, b1):
                    t0 = b * KT                 # global tile index
                    lt0 = t0 - tb0              # tile index within chunk
                    iota_b = AP(iota_ap.tensor, iota_ap.offset,
                                [iota_ap.ap[0], [0, KT], iota_ap.ap[1]])
                    sel_eq = selp.tile([128, KT, R], mybir.dt.float16, tag="se")
                    sel0 = selp.tile([128, KT, R], mybir.dt.float16, tag="s0")
                    sel1 = selp.tile([128, KT, R], mybir.dt.float16, tag="s1")
                    nc.vector.tensor_tensor(
                        out=sel_eq[:],
                        in0=rowl_t[:, t0:t0 + KT].to_broadcast([128, KT, R]),
                        in1=iota_b, op=mybir.AluOpType.is_equal)
                    nc.vector.tensor_tensor(
                        out=sel0[:], in0=sel_eq[:],
                        in1=w0_t[:, t0:t0 + KT].to_broadcast([128, KT, R]),
                        op=mybir.AluOpType.mult)
                    nc.vector.tensor_tensor(
                        out=sel1[:], in0=sel_eq[:],
                        in1=w1_t[:, t0:t0 + KT].to_broadcast([128, KT, R]),
                        op=mybir.AluOpType.mult)
                    so = (b - b0) * R
                    for k in range(KT):
                        nc.tensor.matmul(out=ps0[:, so:so + R],
                                         lhsT=g_t[:, lt0 + k, 0:D],
                                         rhs=sel0[:, k, :],
                                         start=(k == 0), stop=(k == KT - 1))
                        nc.tensor.matmul(out=ps1[:, so:so + R],
                                         lhsT=g_t[:, lt0 + k, D:2 * D],
                                         rhs=sel1[:, k, :],
                                         start=(k == 0), stop=(k == KT - 1))

                stage = stp.tile([128, nbc * R], mybir.dt.float32, tag="st")
                if skip_compute:
                    nc.vector.memset(stage[:], 0.0)
                elif fold:
                    nc.scalar.copy(out=stage[:], in_=psf[:])
                else:
                    nc.scalar.copy(out=stage[0:64, :], in_=ps0[:])
                    nc.scalar.copy(out=stage[64:128, :], in_=ps1[:])
                nc.sync.dma_start(out=out_local[:, b0 * R: b1 * R], in_=stage[:])

    nc.compile()
    return nc


def _prepare(H_, edge_index, edge_values, weight):
    """Host-side preprocessing. Returns (nb, in_maps, gidx) where gidx maps
    each core's output columns to global destination rows."""
    H_ = np.asarray(H_, dtype=np.float32)
    edge_index = np.asarray(edge_index)
    edge_values = np.asarray(edge_values, dtype=np.float32)
    weight = np.asarray(weight, dtype=np.float64)

    # softmax over edge types per channel
    wexp = np.exp(weight - weight.max(axis=1, keepdims=True))
    filt = (wexp / wexp.sum(axis=1, keepdims=True)).astype(np.float32)  # [C,T]

    row = np.ascontiguousarray(edge_index[:, 0, :]).reshape(-1).astype(np.int64)
    col = np.ascontiguousarray(edge_index[:, 1, :]).reshape(-1).astype(np.int64)
    ev = edge_values.reshape(-1)
    tt_of_edge = np.repeat(np.arange(T), E)
    wc = filt[:, tt_of_edge] * ev[None, :]      # [C, M]

    H_pre = np.ascontiguousarray(
        np.transpose(H_, (1, 0, 2)).reshape(N, C * D)).astype(np.float16)

    perm = np.argsort(row, kind="stable")
    row_s = row[perm]
    col_s = col[perm]
    w0_s = wc[0][perm].astype(np.float16)
    w1_s = wc[1][perm].astype(np.float16)
    ratio = (filt[1] / filt[0])[tt_of_edge]
    rt_s = ratio[perm].astype(np.float16)
    core_s = row_s // RPC
    rl_s = row_s - core_s * RPC                 # row local to core [0, RPC)

    # greedy window segmentation per core
    core_starts = np.searchsorted(core_s, np.arange(NCORES + 1))
    win_edge0 = [[] for _ in range(NCORES)]     # first edge index of window
    win_base = [[] for _ in range(NCORES)]      # base row (core-local)
    for k in range(NCORES):
        lo, hi = int(core_starts[k]), int(core_starts[k + 1])
        i = lo
        while i < hi:
            base = rl_s[i]
            j = np.searchsorted(rl_s[lo:hi], base + R, side="left") + lo
            j = min(j, i + K)
            win_edge0[k].append(i)
            win_base[k].append(int(base))
            i = j
        win_edge0[k].append(hi)
    nb = max(len(wb) for wb in win_base)

    nslots = nb * K
    idx16 = np.full((NCORES, nslots), PADCOL - BIAS, dtype=np.int16)
    rowl_a = np.full((NCORES, nslots), PADROW, dtype=np.float16)
    w0_a = np.zeros((NCORES, nslots), dtype=np.float16)
    w1_a = np.zeros((NCORES, nslots), dtype=np.float16)
    rt_a = np.zeros((NCORES, nslots), dtype=np.float16)
    gidx = np.zeros((NCORES, nslots // KT // 128 * R), dtype=np.int64)

    for k in range(NCORES):
        e0 = np.asarray(win_edge0[k][:-1], dtype=np.int64)
        e1 = np.asarray(win_edge0[k][1:], dtype=np.int64)
        bases = np.asarray(win_base[k], dtype=np.int64)
        nwk = len(bases)
        cnt = e1 - e0
        # slot for edge i of window b: b*K + (i - e0[b])
        wid = np.repeat(np.arange(nwk), cnt)
        ed = np.arange(int(e0[0]) if nwk else 0, int(e1[-1]) if nwk else 0)
        slot = wid * K + (ed - e0[wid])
        idx16[k, slot] = (col_s[ed] - BIAS).astype(np.int16)
        rowl_a[k, slot] = (rl_s[ed] - bases[wid]).astype(np.float16)
        w0_a[k, slot] = w0_s[ed]
        w1_a[k, slot] = w1_s[ed]
        rt_a[k, slot] = rt_s[ed]
        # output column -> global row (clamp pads inside range; they add 0)
        gb = np.minimum(bases[:, None] + np.arange(R)[None, :], RPC - 1)
        gidx[k, :nwk * R] = (gb + k * RPC).reshape(-1)
        gidx[k, nwk * R:] = k * RPC  # unused windows: psum cols are zero

    # ensure the LAST slot of every gather chunk has idx >= 0 (dma_gather
    # trims a trailing negative run); swap within the final tile if needed
    nchunks = (nb + CB - 1) // CB
    for cidx in range(nchunks):
        b1 = min((cidx + 1) * CB, nb)
        end = b1 * K                 # one past chunk's last slot
        for k in range(NCORES):
            if idx16[k, end - 1] < 0:
                tile_lo = end - 128
                cand = np.nonzero(idx16[k, tile_lo:end - 1] >= 0)[0]
                assert cand.size > 0, "entire tile has negative idx"
                j = tile_lo + cand[-1]
                for arr in (idx16, rowl_a, w0_a, w1_a, rt_a):
                    arr[k, j], arr[k, end - 1] = arr[k, end - 1], arr[k, j]

    tt = nb * KT
    iota_np = np.tile(np.arange(R, dtype=np.float16), (128, 1))
    in_maps = []
    for k in range(NCORES):
        in_maps.append({
            "hpre": H_pre,
            # idx position q -> partition q%16, free q//16; replicate x8
            "idx": np.ascontiguousarray(
                np.tile(idx16[k].reshape(nslots // 16, 16).T, (8, 1))),
            "rowl": np.ascontiguousarray(rowl_a[k].reshape(tt, 128).T),
            "w0": np.ascontiguousarray(w0_a[k].reshape(tt, 128).T),
            "w1": np.ascontiguousarray(w1_a[k].reshape(tt, 128).T),
            "rt": np.ascontiguousarray(rt_a[k].reshape(tt, 128).T),
            "iota": iota_np,
        })
    return nb, in_maps, gidx


def _make_runner(nc):
    """Build and cache a jitted shard_map executor for the compiled program."""
    import jax
    from jax.sharding import Mesh, PartitionSpec, NamedSharding
    from jax.experimental.shard_map import shard_map
    from concourse import mybir
    from concourse.bass2jax import (_bass_exec_p, partition_id_tensor,
                                    install_neuronx_cc_hook)

    install_neuronx_cc_hook()
    partition_name = nc.partition_id_tensor.name if nc.partition_id_tensor else None
    in_names, out_names, out_avals = [], [], []
    for alloc in nc.m.functions[0].allocations:
        if not isinstance(alloc, mybir.MemoryLocationSet):
            continue
        name = alloc.memorylocations[0].name
        if alloc.kind == "ExternalInput":
            if name != partition_name:
                in_names.append(name)
        elif alloc.kind == "ExternalOutput":
            out_names.append(name)
            out_avals.append(jax.core.ShapedArray(
                tuple(alloc.tensor_shape), mybir.dt.np(alloc.dtype)))
    n_params = len(in_names)
    all_in = in_names + out_names + ([partition_name] if partition_name else [])

    def _body(*args):
        operands = list(args)
        if partition_name is not None:
            operands.append(partition_id_tensor())
        return tuple(_bass_exec_p.bind(
            *operands, out_avals=tuple(out_avals), in_names=tuple(all_in),
            out_names=tuple(out_names), lowering_input_output_aliases=(),
            sim_require_finite=True, sim_require_nnan=True, nc=nc))

    devices = jax.devices()[:NCORES]
    mesh = Mesh(np.asarray(devices), ("core",))
    spec = PartitionSpec("core")
    f = jax.jit(shard_map(_body, mesh=mesh,
                          in_specs=(spec,) * (n_params + len(out_names)),
                          out_specs=(spec,), check_rep=False))
    sharding = NamedSharding(mesh, spec)
    zeros = [np.zeros((av.shape[0] * NCORES,) + av.shape[1:], av.dtype)
             for av in out_avals]
    return {"f": f, "in_names": in_names, "out_names": out_names,
            "sharding": sharding, "zeros": zeros}


def kernel(H_, edge_index, edge_values, weight, num_nodes):
    import jax

    nb, in_maps, gidx = _prepare(H_, edge_index, edge_values, weight)
    if nb not in _prog_cache:
        nc = _build_program(nb, **PROG_KW)
        _prog_cache[nb] = _make_runner(nc)
    rn = _prog_cache[nb]

    args = []
    for name in rn["in_names"]:
        glob = np.concatenate([m[name] for m in in_maps], axis=0)
        args.append(jax.device_put(glob, rn["sharding"]))
    for z in rn["zeros"]:
        args.append(jax.device_put(z, rn["sharding"]))
    outs = rn["f"](*args)
    res = np.asarray(outs[rn["out_names"].index("out_local")])  # [8*128, nb*R]

    out = np.zeros((C, N, D), dtype=np.float32)
    for k in range(NCORES):
        ol = res[k * 128:(k + 1) * 128]          # [128, nb*R]
        np.add.at(out[0], gidx[k], ol[0:D].T)
        np.add.at(out[1], gidx[k], ol[D:2 * D].T)
    return out


# revision 3
# speedup vs baseline: 1.1685x; 1.1685x over previous
"""FastGTLayer GNN message passing on 8 Trainium2 NeuronCores — v2.

Strategy (destination-window sharded, fp16 gather + selection-matmul scatter):
- Host: softmax(weight) -> per-edge per-channel weights w_c = filt[c,t]*ev[t,e].
  Edges sharded by destination row range (6250 rows/core) and sorted by row.
  Each core's sorted edge list is segmented greedily into "windows": at most
  KT*128 edges AND spanning at most R=32 destination rows.  Every window gets
  exactly KT 128-edge tiles (tail padded with inert slots), so the device
  program is fully uniform — no per-block quota vector, ~0.5% padding
  (vs ~12% for per-block max-over-cores quotas).
- Device (SPMD, one program on 8 cores): for each chunk of CB windows,
  dma_gather fetches H rows (both channels interleaved, 256B/edge fp16) by
  int16 index with a biased base; DVE builds weighted one-hot selection
  matrices sel_c[e,r] = w_c[e]*(rowl[e]==r) in fp16; PE scatter-adds via
  fp16 matmul psum_c[64f, 32r] += g_c^T @ sel_c (4x the fp32 matmul rate).
  PSB consecutive windows accumulate into one [64, PSB*32] psum tile per
  channel; ACT evicts whole groups; HWDGE writes [128,(c,d)] x cols to HBM.
- Host: windows of core k map to row ranges [base(k,b), base(k,b)+32); the
  unshard scatter-adds window columns into out[C, N, D] (windows may split
  one row across two windows; np.add.at handles the overlap).
"""
import sys
if "/opt/trn_rl_repo" not in sys.path:
    sys.path.insert(0, "/opt/trn_rl_repo")

import numpy as np

C, T, N, E, D = 2, 4, 50000, 400000, 64
M = T * E
NCORES = 8
RPC = N // NCORES          # 6250 destination rows per core
R = 32                     # max rows per window (psum window)
KT = 7                     # tiles (x128 edges) per window
K = KT * 128               # max edges per window
BIAS = N - 32768           # 17232; idx = col - BIAS in [-17232, 32767]
PADCOL = 40000             # pad slots gather this row (positive idx), weight 0
CB = 14                    # windows per gather chunk / psum group
PADROW = 33.0              # pad rowl value: never equals iota 0..31

PROG_KW = dict(fold=True, gbufs=5, selbufs=2)

_prog_cache = {}


def _build_program(nb, nqueues=4, gbufs=3, selbufs=3, pbufs=4, sbufs=2,
                   skip_gather=False, skip_compute=False, tiny_gather=False,
                   fold=False, no_ratio=False, no_mm=False,
                   scratch=16384, repeat=1, cb=None):
    """Build the SPMD Bass program for `nb` windows of KT tiles each."""
    from concourse import bacc, mybir
    import concourse.tile as tile
    from concourse.bass import AP

    cb = cb or CB
    tt = nb * KT
    nc = bacc.Bacc("TRN2", num_swdge_queues=nqueues, dynamic_dma_scratch_size=scratch)
    hpre = nc.dram_tensor("hpre", [N, 2 * D], mybir.dt.float16, kind="ExternalInput")
    idx = nc.dram_tensor("idx", [128, tt * 8], mybir.dt.int16, kind="ExternalInput")
    rowl = nc.dram_tensor("rowl", [128, tt], mybir.dt.float16, kind="ExternalInput")
    w0 = nc.dram_tensor("w0", [128, tt], mybir.dt.float16, kind="ExternalInput")
    w1 = nc.dram_tensor("rt" if fold else "w1", [128, tt], mybir.dt.float16,
                        kind="ExternalInput")
    iota = nc.dram_tensor("iota", [128, R], mybir.dt.float16, kind="ExternalInput")
    out_local = nc.dram_tensor("out_local", [128, nb * R], mybir.dt.float32,
                               kind="ExternalOutput")

    nchunks = (nb + cb - 1) // cb

    with tile.TileContext(nc) as tc:
        with tc.tile_pool(name="meta", bufs=1) as mp, \
             tc.tile_pool(name="gp", bufs=gbufs) as gp, \
             tc.tile_pool(name="selp", bufs=selbufs) as selp, \
             tc.tile_pool(name="stp", bufs=sbufs) as stp, \
             tc.tile_pool(name="pp", bufs=pbufs, space="PSUM") as pp:
            idx_t = mp.tile([128, tt * 8], mybir.dt.int16)
            rowl_t = mp.tile([128, tt], mybir.dt.float16)
            w0_t = mp.tile([128, tt], mybir.dt.float16)
            w1_t = mp.tile([128, tt], mybir.dt.float16)
            iota_t = mp.tile([128, R], mybir.dt.float16)

            nc.gpsimd.dma_start(out=idx_t[:], in_=idx[:])
            nc.gpsimd.dma_start(out=rowl_t[:], in_=rowl[:])
            nc.gpsimd.dma_start(out=w0_t[:], in_=w0[:])
            nc.gpsimd.dma_start(out=w1_t[:], in_=w1[:])
            nc.gpsimd.dma_start(out=iota_t[:], in_=iota[:])

            iota_ap = iota_t[:]

            for rep in range(repeat):
              for c in range(nchunks):
                b0 = c * cb
                b1 = min(b0 + cb, nb)
                nbc = b1 - b0                # windows in this chunk
                ct = nbc * KT                # tiles in this chunk
                nidx = ct * 128
                tb0 = b0 * KT                # first global tile of chunk

                g_t = gp.tile([128, ct, 2 * D], mybir.dt.float16, tag="g")
                if not skip_gather:
                    g_nidx = 128 if tiny_gather else nidx
                    g_out = g_t[:, 0:1, :] if tiny_gather else g_t[:]
                    nc.gpsimd.dma_gather(
                        g_out,
                        hpre[BIAS:, :],
                        idx_t[:, tb0 * 8: tb0 * 8 + g_nidx // 16],
                        g_nidx,
                        g_nidx,
                        2 * D,
                        queue_num=(c % nqueues),
                        single_packet=False,
                    )

                if fold and not skip_compute and not skip_gather and not no_ratio:
                    # fold channel-1 weights into gathered data:
                    # g1' = g1 * (w1/w0); both channels then share sel0
                    nc.vector.tensor_tensor(
                        out=g_t[:, :, D:2 * D], in0=g_t[:, :, D:2 * D],
                        in1=w1_t[:, tb0:tb0 + ct].to_broadcast([128, ct, D]),
                        op=mybir.AluOpType.mult)

                if no_mm:
                    stage = stp.tile([128, nbc * R], mybir.dt.float32, tag="st")
                    nc.vector.memset(stage[:], 0.0)
                    nc.sync.dma_start(out=out_local[:, b0 * R: b1 * R],
                                      in_=stage[:])
                    continue

                if fold and not skip_compute:
                    # chunk-wide selection build: 2 DVE ops for all ct tiles
                    # (in-place: sel0 = (rowl==iota); sel0 *= w0)
                    psf = pp.tile([128, nbc * R], mybir.dt.float32, space="PSUM",
                                  tag="pf")
                    iota_c = AP(iota_ap.tensor, iota_ap.offset,
                                [iota_ap.ap[0], [0, ct], iota_ap.ap[1]])
                    sel0 = selp.tile([128, ct, R], mybir.dt.float16, tag="s0")
                    nc.vector.tensor_tensor(
                        out=sel0[:],
                        in0=rowl_t[:, tb0:tb0 + ct].to_broadcast([128, ct, R]),
                        in1=iota_c, op=mybir.AluOpType.is_equal)
                    nc.vector.tensor_tensor(
                        out=sel0[:], in0=sel0[:],
                        in1=w0_t[:, tb0:tb0 + ct].to_broadcast([128, ct, R]),
                        op=mybir.AluOpType.mult)
                    for b in range(b0, b1):
                        lt0 = b * KT - tb0
                        so = (b - b0) * R
                        for k in range(KT):
                            nc.tensor.matmul(out=psf[:, so:so + R],
                                             lhsT=g_t[:, lt0 + k, :],
                                             rhs=sel0[:, lt0 + k, :],
                                             start=(k == 0), stop=(k == KT - 1))
                elif not skip_compute:
                    ps0 = pp.tile([64, nbc * R], mybir.dt.float32, space="PSUM",
                                  tag="p0")
                    ps1 = pp.tile([64, nbc * R], mybir.dt.float32, space="PSUM",
                                  tag="p1")
                for b in [] if (skip_compute or fold) else range(b0, b1):
                    t0 = b * KT                 # global tile index
                    lt0 = t0 - tb0              # tile index within chunk
                    iota_b = AP(iota_ap.tensor, iota_ap.offset,
                                [iota_ap.ap[0], [0, KT], iota_ap.ap[1]])
                    sel_eq = selp.tile([128, KT, R], mybir.dt.float16, tag="se")
                    sel0 = selp.tile([128, KT, R], mybir.dt.float16, tag="s0")
                    sel1 = selp.tile([128, KT, R], mybir.dt.float16, tag="s1")
                    nc.vector.tensor_tensor(
                        out=sel_eq[:],
                        in0=rowl_t[:, t0:t0 + KT].to_broadcast([128, KT, R]),
                        in1=iota_b, op=mybir.AluOpType.is_equal)
                    nc.vector.tensor_tensor(
                        out=sel0[:], in0=sel_eq[:],
                        in1=w0_t[:, t0:t0 + KT].to_broadcast([128, KT, R]),
                        op=mybir.AluOpType.mult)
                    nc.vector.tensor_tensor(
                        out=sel1[:], in0=sel_eq[:],
                        in1=w1_t[:, t0:t0 + KT].to_broadcast([128, KT, R]),
                        op=mybir.AluOpType.mult)
                    so = (b - b0) * R
                    for k in range(KT):
                        nc.tensor.matmul(out=ps0[:, so:so + R],
                                         lhsT=g_t[:, lt0 + k, 0:D],
                                         rhs=sel0[:, k, :],
                                         start=(k == 0), stop=(k == KT - 1))
                        nc.tensor.matmul(out=ps1[:, so:so + R],
                                         lhsT=g_t[:, lt0 + k, D:2 * D],
                                         rhs=sel1[:, k, :],
                                         start=(k == 0), stop=(k == KT - 1))

                stage = stp.tile([128, nbc * R], mybir.dt.float32, tag="st")
                if skip_compute:
                    nc.vector.memset(stage[:], 0.0)
                elif fold:
                    nc.scalar.copy(out=stage[:], in_=psf[:])
                else:
                    nc.scalar.copy(out=stage[0:64, :], in_=ps0[:])
                    nc.scalar.copy(out=stage[64:128, :], in_=ps1[:])
                nc.sync.dma_start(out=out_local[:, b0 * R: b1 * R], in_=stage[:])

    nc.compile()
    return nc


def _prepare(H_, edge_index, edge_values, weight):
    """Host-side preprocessing. Returns (nb, in_maps, gidx) where gidx maps
    each core's output columns to global destination rows."""
    H_ = np.asarray(H_, dtype=np.float32)
    edge_index = np.asarray(edge_index)
    edge_values = np.asarray(edge_values, dtype=np.float32)
    weight = np.asarray(weight, dtype=np.float64)

    # softmax over edge types per channel
    wexp = np.exp(weight - weight.max(axis=1, keepdims=True))
    filt = (wexp / wexp.sum(axis=1, keepdims=True)).astype(np.float32)  # [C,T]

    row = np.ascontiguousarray(edge_index[:, 0, :]).reshape(-1).astype(np.int64)
    col = np.ascontiguousarray(edge_index[:, 1, :]).reshape(-1).astype(np.int64)
    ev = edge_values.reshape(-1)
    tt_of_edge = np.repeat(np.arange(T), E)
    wc = filt[:, tt_of_edge] * ev[None, :]      # [C, M]

    H_pre = np.ascontiguousarray(
        np.transpose(H_, (1, 0, 2)).reshape(N, C * D)).astype(np.float16)

    perm = np.argsort(row, kind="stable")
    row_s = row[perm]
    col_s = col[perm]
    w0_s = wc[0][perm].astype(np.float16)
    w1_s = wc[1][perm].astype(np.float16)
    ratio = (filt[1] / filt[0])[tt_of_edge]
    rt_s = ratio[perm].astype(np.float16)
    core_s = row_s // RPC
    rl_s = row_s - core_s * RPC                 # row local to core [0, RPC)

    # greedy window segmentation per core
    core_starts = np.searchsorted(core_s, np.arange(NCORES + 1))
    win_edge0 = [[] for _ in range(NCORES)]     # first edge index of window
    win_base = [[] for _ in range(NCORES)]      # base row (core-local)
    for k in range(NCORES):
        lo, hi = int(core_starts[k]), int(core_starts[k + 1])
        i = lo
        while i < hi:
            base = rl_s[i]
            j = np.searchsorted(rl_s[lo:hi], base + R, side="left") + lo
            j = min(j, i + K)
            win_edge0[k].append(i)
            win_base[k].append(int(base))
            i = j
        win_edge0[k].append(hi)
    nb = max(len(wb) for wb in win_base)

    nslots = nb * K
    idx16 = np.full((NCORES, nslots), PADCOL - BIAS, dtype=np.int16)
    rowl_a = np.full((NCORES, nslots), PADROW, dtype=np.float16)
    w0_a = np.zeros((NCORES, nslots), dtype=np.float16)
    w1_a = np.zeros((NCORES, nslots), dtype=np.float16)
    rt_a = np.zeros((NCORES, nslots), dtype=np.float16)
    gidx = np.zeros((NCORES, nslots // KT // 128 * R), dtype=np.int64)

    for k in range(NCORES):
        e0 = np.asarray(win_edge0[k][:-1], dtype=np.int64)
        e1 = np.asarray(win_edge0[k][1:], dtype=np.int64)
        bases = np.asarray(win_base[k], dtype=np.int64)
        nwk = len(bases)
        cnt = e1 - e0
        # slot for edge i of window b: b*K + (i - e0[b])
        wid = np.repeat(np.arange(nwk), cnt)
        ed = np.arange(int(e0[0]) if nwk else 0, int(e1[-1]) if nwk else 0)
        slot = wid * K + (ed - e0[wid])
        idx16[k, slot] = (col_s[ed] - BIAS).astype(np.int16)
        rowl_a[k, slot] = (rl_s[ed] - bases[wid]).astype(np.float16)
        w0_a[k, slot] = w0_s[ed]
        w1_a[k, slot] = w1_s[ed]
        rt_a[k, slot] = rt_s[ed]
        # output column -> global row (clamp pads inside range; they add 0)
        gb = np.minimum(bases[:, None] + np.arange(R)[None, :], RPC - 1)
        gidx[k, :nwk * R] = (gb + k * RPC).reshape(-1)
        gidx[k, nwk * R:] = k * RPC  # unused windows: psum cols are zero

    # ensure the LAST slot of every gather chunk has idx >= 0 (dma_gather
    # trims a trailing negative run); swap within the final tile if needed
    nchunks = (nb + CB - 1) // CB
    for cidx in range(nchunks):
        b1 = min((cidx + 1) * CB, nb)
        end = b1 * K                 # one past chunk's last slot
        for k in range(NCORES):
            if idx16[k, end - 1] < 0:
                tile_lo = end - 128
                cand = np.nonzero(idx16[k, tile_lo:end - 1] >= 0)[0]
                assert cand.size > 0, "entire tile has negative idx"
                j = tile_lo + cand[-1]
                for arr in (idx16, rowl_a, w0_a, w1_a, rt_a):
                    arr[k, j], arr[k, end - 1] = arr[k, end - 1], arr[k, j]

    tt = nb * KT
    iota_np = np.tile(np.arange(R, dtype=np.float16), (128, 1))
    in_maps = []
    for k in range(NCORES):
        in_maps.append({
            "hpre": H_pre,
            # idx position q -> partition q%16, free q//16; replicate x8
            "idx": np.ascontiguousarray(
                np.tile(idx16[k].reshape(nslots // 16, 16).T, (8, 1))),
            "rowl": np.ascontiguousarray(rowl_a[k].reshape(tt, 128).T),
            "w0": np.ascontiguousarray(w0_a[k].reshape(tt, 128).T),
            "w1": np.ascontiguousarray(w1_a[k].reshape(tt, 128).T),
            "rt": np.ascontiguousarray(rt_a[k].reshape(tt, 128).T),
            "iota": iota_np,
        })
    return nb, in_maps, gidx


def _make_runner(nc):
    """Build and cache a jitted shard_map executor for the compiled program."""
    import jax
    from jax.sharding import Mesh, PartitionSpec, NamedSharding
    from jax.experimental.shard_map import shard_map
    from concourse import mybir
    from concourse.bass2jax import (_bass_exec_p, partition_id_tensor,
                                    install_neuronx_cc_hook)

    install_neuronx_cc_hook()
    partition_name = nc.partition_id_tensor.name if nc.partition_id_tensor else None
    in_names, out_names, out_avals = [], [], []
    for alloc in nc.m.functions[0].allocations:
        if not isinstance(alloc, mybir.MemoryLocationSet):
            continue
        name = alloc.memorylocations[0].name
        if alloc.kind == "ExternalInput":
            if name != partition_name:
                in_names.append(name)
        elif alloc.kind == "ExternalOutput":
            out_names.append(name)
            out_avals.append(jax.core.ShapedArray(
                tuple(alloc.tensor_shape), mybir.dt.np(alloc.dtype)))
    n_params = len(in_names)
    all_in = in_names + out_names + ([partition_name] if partition_name else [])

    def _body(*args):
        operands = list(args)
        if partition_name is not None:
            operands.append(partition_id_tensor())
        return tuple(_bass_exec_p.bind(
            *operands, out_avals=tuple(out_avals), in_names=tuple(all_in),
            out_names=tuple(out_names), lowering_input_output_aliases=(),
            sim_require_finite=True, sim_require_nnan=True, nc=nc))

    devices = jax.devices()[:NCORES]
    mesh = Mesh(np.asarray(devices), ("core",))
    spec = PartitionSpec("core")
    f = jax.jit(shard_map(_body, mesh=mesh,
                          in_specs=(spec,) * (n_params + len(out_names)),
                          out_specs=(spec,), check_rep=False))
    sharding = NamedSharding(mesh, spec)
    zeros = [np.zeros((av.shape[0] * NCORES,) + av.shape[1:], av.dtype)
             for av in out_avals]
    return {"f": f, "in_names": in_names, "out_names": out_names,
            "sharding": sharding, "zeros": zeros}


def kernel(H_, edge_index, edge_values, weight, num_nodes):
    import jax

    nb, in_maps, gidx = _prepare(H_, edge_index, edge_values, weight)
    if nb not in _prog_cache:
        nc = _build_program(nb, **PROG_KW)
        _prog_cache[nb] = _make_runner(nc)
    rn = _prog_cache[nb]

    args = []
    for name in rn["in_names"]:
        glob = np.concatenate([m[name] for m in in_maps], axis=0)
        args.append(jax.device_put(glob, rn["sharding"]))
    for z in rn["zeros"]:
        args.append(jax.device_put(z, rn["sharding"]))
    outs = rn["f"](*args)
    res = np.asarray(outs[rn["out_names"].index("out_local")])  # [8*128, nb*R]

    out = np.zeros((C, N, D), dtype=np.float32)
    for k in range(NCORES):
        ol = res[k * 128:(k + 1) * 128]          # [128, nb*R]
        np.add.at(out[0], gidx[k], ol[0:D].T)
        np.add.at(out[1], gidx[k], ol[D:2 * D].T)
    return out


# revision 5
# speedup vs baseline: 1.2981x; 1.1109x over previous
"""FastGTLayer GNN message passing on 8 Trainium2 NeuronCores.

Strategy (destination-window sharded, fp16 gather + selection-matmul scatter):
- Host: softmax(weight) -> per-edge channel-0 weights w0 = filt[0,t]*ev[t,e]
  and the channel ratio rt = filt[1,t]/filt[0,t] (type-only).  Edges are
  sharded by destination row range (6250 rows/core) and sorted by row.  Each
  core's sorted edge list is segmented greedily into "windows": at most
  KT*128 edges AND spanning at most R=32 destination rows.  Every window gets
  exactly KT 128-edge tiles (tail padded with inert slots), so the device
  program is fully uniform -- no per-block quota vector, ~0.5% padding
  (vs ~12% for per-block max-over-cores quotas).
- Device (SPMD, one program on 8 cores): for each chunk of CB=14 windows,
  dma_gather fetches H rows (both channels interleaved, 256B/edge fp16) by
  int16 index with a biased base (idx = col - BIAS in [-17232, 32767]);
  one DVE op rescales the gathered channel-1 half by rt so both channels
  share one selection matrix; two chunk-wide DVE ops build it in place:
  sel[e,r] = w0[e]*(rowl[e]==r) in fp16; PE scatter-adds via one fp16 matmul
  per tile psum[128cd, 32r] += g^T @ sel (fp16 = 1 PE cycle/row vs 4 for
  fp32); ACT evicts one [128, CB*32] psum tile per chunk; HWDGE writes the
  staged chunk to HBM.  gbufs=5 gather buffers (> 4 SWDGE queues) keep the
  in-order Pool engine from stalling, which fully hides compute under the
  gather (the kernel is gather-descriptor-bound at ~2-2.5 ns/descriptor).
- Host: window b of core k covers rows [base(k,b), base(k,b)+32); the
  unshard scatter-adds window columns into out[C, N, D] via np.add.at
  (a row split across two windows gets partial sums from both).

Correctness gotcha encoded in _prepare: dma_gather ignores a trailing
negative-idx run per instruction, leaving those slots as uninitialized SBUF
(NaN x 0 = NaN in the matmul), so the last slot of every CB-window chunk is
swapped to a non-negative idx.  Chunk boundaries in _prepare and
_build_program must both derive from CB.
"""
import sys
if "/opt/trn_rl_repo" not in sys.path:
    sys.path.insert(0, "/opt/trn_rl_repo")

import numpy as np

C, T, N, E, D = 2, 4, 50000, 400000, 64
M = T * E
NCORES = 8
RPC = N // NCORES          # 6250 destination rows per core
R = 32                     # max rows per window (psum window)
KT = 7                     # tiles (x128 edges) per window
K = KT * 128               # max edges per window
BIAS = N - 32768           # 17232; idx = col - BIAS in [-17232, 32767]
PADCOL = 40000             # pad slots gather this row (positive idx), weight 0
CB = 14                    # windows per gather chunk / psum group
PADROW = 33.0              # pad rowl value: never equals iota 0..31

PROG_KW = dict(fold=True, gbufs=5, selbufs=2)

_prog_cache = {}


def _chunk_bounds(nb, cb=None, ramp=False):
    """Chunk the nb windows into gather-instruction groups (uniform cb-sized
    chunks).  ramp=True splits the first/last 2*cb windows into half-size
    chunks for faster fill/drain, but measured WORSE (-11%): the extra
    per-instruction SWDGE overhead beats the ramp savings.  Used by BOTH
    _prepare and _build_program -- the trailing-negative-idx swap must
    target these exact boundaries."""
    cb = cb or CB
    half = cb // 2
    sizes = []
    if ramp and nb >= 6 * half:
        sizes += [half] * 4
        rem = nb - 8 * half
        sizes += [cb] * (rem // cb)
        if rem % cb:
            sizes.append(rem % cb)
        sizes += [half] * 4
    else:
        rem = nb
        while rem > 0:
            sizes.append(min(cb, rem))
            rem -= cb
    bounds = [0]
    for s in sizes:
        bounds.append(bounds[-1] + s)
    assert bounds[-1] == nb
    return bounds


def _build_program(nb, nqueues=4, gbufs=3, selbufs=3, pbufs=4, sbufs=2,
                   skip_gather=False, skip_compute=False, tiny_gather=False,
                   fold=False, no_ratio=False, no_mm=False, ramp=False,
                   scratch=16384, repeat=1, cb=None):
    """Build the SPMD Bass program for `nb` windows of KT tiles each."""
    from concourse import bacc, mybir
    import concourse.tile as tile
    from concourse.bass import AP

    cb = cb or CB
    tt = nb * KT
    nc = bacc.Bacc("TRN2", num_swdge_queues=nqueues, dynamic_dma_scratch_size=scratch)
    hpre = nc.dram_tensor("hpre", [N, 2 * D], mybir.dt.float16, kind="ExternalInput")
    idx = nc.dram_tensor("idx", [128, tt * 8], mybir.dt.int16, kind="ExternalInput")
    rowl = nc.dram_tensor("rowl", [128, tt], mybir.dt.float16, kind="ExternalInput")
    w0 = nc.dram_tensor("w0", [128, tt], mybir.dt.float16, kind="ExternalInput")
    w1 = nc.dram_tensor("rt" if fold else "w1", [128, tt], mybir.dt.float16,
                        kind="ExternalInput")
    iota = nc.dram_tensor("iota", [128, R], mybir.dt.float16, kind="ExternalInput")
    out_local = nc.dram_tensor("out_local", [128, nb * R], mybir.dt.float32,
                               kind="ExternalOutput")

    cbounds = _chunk_bounds(nb, cb, ramp=ramp)
    nchunks = len(cbounds) - 1

    with tile.TileContext(nc) as tc:
        with tc.tile_pool(name="meta", bufs=1) as mp, \
             tc.tile_pool(name="gp", bufs=gbufs) as gp, \
             tc.tile_pool(name="selp", bufs=selbufs) as selp, \
             tc.tile_pool(name="stp", bufs=sbufs) as stp, \
             tc.tile_pool(name="pp", bufs=pbufs, space="PSUM") as pp:
            idx_t = mp.tile([128, tt * 8], mybir.dt.int16)
            rowl_t = mp.tile([128, tt], mybir.dt.float16)
            w0_t = mp.tile([128, tt], mybir.dt.float16)
            w1_t = mp.tile([128, tt], mybir.dt.float16)
            iota_t = mp.tile([128, R], mybir.dt.float16)

            nc.gpsimd.dma_start(out=idx_t[:], in_=idx[:])
            nc.gpsimd.dma_start(out=rowl_t[:], in_=rowl[:])
            nc.gpsimd.dma_start(out=w0_t[:], in_=w0[:])
            nc.gpsimd.dma_start(out=w1_t[:], in_=w1[:])
            nc.gpsimd.dma_start(out=iota_t[:], in_=iota[:])

            iota_ap = iota_t[:]

            for rep in range(repeat):
              for c in range(nchunks):
                b0 = cbounds[c]
                b1 = cbounds[c + 1]
                nbc = b1 - b0                # windows in this chunk
                ct = nbc * KT                # tiles in this chunk
                nidx = ct * 128
                tb0 = b0 * KT                # first global tile of chunk

                g_t = gp.tile([128, ct, 2 * D], mybir.dt.float16, tag="g")
                if not skip_gather:
                    g_nidx = 128 if tiny_gather else nidx
                    g_out = g_t[:, 0:1, :] if tiny_gather else g_t[:]
                    nc.gpsimd.dma_gather(
                        g_out,
                        hpre[BIAS:, :],
                        idx_t[:, tb0 * 8: tb0 * 8 + g_nidx // 16],
                        g_nidx,
                        g_nidx,
                        2 * D,
                        queue_num=(c % nqueues),
                        single_packet=False,
                    )

                if fold and not skip_compute and not skip_gather and not no_ratio:
                    # fold channel-1 weights into gathered data:
                    # g1' = g1 * (w1/w0); both channels then share sel0
                    nc.vector.tensor_tensor(
                        out=g_t[:, :, D:2 * D], in0=g_t[:, :, D:2 * D],
                        in1=w1_t[:, tb0:tb0 + ct].to_broadcast([128, ct, D]),
                        op=mybir.AluOpType.mult)

                if no_mm:
                    stage = stp.tile([128, nbc * R], mybir.dt.float32, tag="st")
                    nc.vector.memset(stage[:], 0.0)
                    nc.sync.dma_start(out=out_local[:, b0 * R: b1 * R],
                                      in_=stage[:])
                    continue

                if fold and not skip_compute:
                    # chunk-wide selection build: 2 DVE ops for all ct tiles
                    # (in-place: sel0 = (rowl==iota); sel0 *= w0)
                    psf = pp.tile([128, nbc * R], mybir.dt.float32, space="PSUM",
                                  tag="pf")
                    iota_c = AP(iota_ap.tensor, iota_ap.offset,
                                [iota_ap.ap[0], [0, ct], iota_ap.ap[1]])
                    sel0 = selp.tile([128, ct, R], mybir.dt.float16, tag="s0")
                    nc.vector.tensor_tensor(
                        out=sel0[:],
                        in0=rowl_t[:, tb0:tb0 + ct].to_broadcast([128, ct, R]),
                        in1=iota_c, op=mybir.AluOpType.is_equal)
                    nc.vector.tensor_tensor(
                        out=sel0[:], in0=sel0[:],
                        in1=w0_t[:, tb0:tb0 + ct].to_broadcast([128, ct, R]),
                        op=mybir.AluOpType.mult)
                    for b in range(b0, b1):
                        lt0 = b * KT - tb0
                        so = (b - b0) * R
                        for k in range(KT):
                            nc.tensor.matmul(out=psf[:, so:so + R],
                                             lhsT=g_t[:, lt0 + k, :],
                                             rhs=sel0[:, lt0 + k, :],
                                             start=(k == 0), stop=(k == KT - 1))
                elif not skip_compute:
                    ps0 = pp.tile([64, nbc * R], mybir.dt.float32, space="PSUM",
                                  tag="p0")
                    ps1 = pp.tile([64, nbc * R], mybir.dt.float32, space="PSUM",
                                  tag="p1")
                for b in [] if (skip_compute or fold) else range(b0, b1):
                    t0 = b * KT                 # global tile index
                    lt0 = t0 - tb0              # tile index within chunk
                    iota_b = AP(iota_ap.tensor, iota_ap.offset,
                                [iota_ap.ap[0], [0, KT], iota_ap.ap[1]])
                    sel_eq = selp.tile([128, KT, R], mybir.dt.float16, tag="se")
                    sel0 = selp.tile([128, KT, R], mybir.dt.float16, tag="s0")
                    sel1 = selp.tile([128, KT, R], mybir.dt.float16, tag="s1")
                    nc.vector.tensor_tensor(
                        out=sel_eq[:],
                        in0=rowl_t[:, t0:t0 + KT].to_broadcast([128, KT, R]),
                        in1=iota_b, op=mybir.AluOpType.is_equal)
                    nc.vector.tensor_tensor(
                        out=sel0[:], in0=sel_eq[:],
                        in1=w0_t[:, t0:t0 + KT].to_broadcast([128, KT, R]),
                        op=mybir.AluOpType.mult)
                    nc.vector.tensor_tensor(
                        out=sel1[:], in0=sel_eq[:],
                        in1=w1_t[:, t0:t0 + KT].to_broadcast([128, KT, R]),
                        op=mybir.AluOpType.mult)
                    so = (b - b0) * R
                    for k in range(KT):
                        nc.tensor.matmul(out=ps0[:, so:so + R],
                                         lhsT=g_t[:, lt0 + k, 0:D],
                                         rhs=sel0[:, k, :],
                                         start=(k == 0), stop=(k == KT - 1))
                        nc.tensor.matmul(out=ps1[:, so:so + R],
                                         lhsT=g_t[:, lt0 + k, D:2 * D],
                                         rhs=sel1[:, k, :],
                                         start=(k == 0), stop=(k == KT - 1))

                stage = stp.tile([128, nbc * R], mybir.dt.float32, tag="st")
                if skip_compute:
                    nc.vector.memset(stage[:], 0.0)
                elif fold:
                    nc.scalar.copy(out=stage[:], in_=psf[:])
                else:
                    nc.scalar.copy(out=stage[0:64, :], in_=ps0[:])
                    nc.scalar.copy(out=stage[64:128, :], in_=ps1[:])
                nc.sync.dma_start(out=out_local[:, b0 * R: b1 * R], in_=stage[:])

    nc.compile()
    return nc


def _prepare(H_, edge_index, edge_values, weight):
    """Host-side preprocessing. Returns (nb, in_maps, gidx) where gidx maps
    each core's output columns to global destination rows."""
    H_ = np.asarray(H_, dtype=np.float32)
    edge_index = np.asarray(edge_index)
    edge_values = np.asarray(edge_values, dtype=np.float32)
    weight = np.asarray(weight, dtype=np.float64)

    # softmax over edge types per channel
    wexp = np.exp(weight - weight.max(axis=1, keepdims=True))
    filt = (wexp / wexp.sum(axis=1, keepdims=True)).astype(np.float32)  # [C,T]

    row = np.ascontiguousarray(edge_index[:, 0, :]).reshape(-1).astype(np.int64)
    col = np.ascontiguousarray(edge_index[:, 1, :]).reshape(-1).astype(np.int64)
    ev = edge_values.reshape(-1)
    tt_of_edge = np.repeat(np.arange(T), E)
    wc = filt[:, tt_of_edge] * ev[None, :]      # [C, M]

    H_pre = np.ascontiguousarray(
        np.transpose(H_, (1, 0, 2)).reshape(N, C * D)).astype(np.float16)

    perm = np.argsort(row, kind="stable")
    row_s = row[perm]
    col_s = col[perm]
    w0_s = wc[0][perm].astype(np.float16)
    w1_s = wc[1][perm].astype(np.float16)
    ratio = (filt[1] / filt[0])[tt_of_edge]
    rt_s = ratio[perm].astype(np.float16)
    core_s = row_s // RPC
    rl_s = row_s - core_s * RPC                 # row local to core [0, RPC)

    # greedy window segmentation per core
    core_starts = np.searchsorted(core_s, np.arange(NCORES + 1))
    win_edge0 = [[] for _ in range(NCORES)]     # first edge index of window
    win_base = [[] for _ in range(NCORES)]      # base row (core-local)
    for k in range(NCORES):
        lo, hi = int(core_starts[k]), int(core_starts[k + 1])
        i = lo
        while i < hi:
            base = rl_s[i]
            j = np.searchsorted(rl_s[lo:hi], base + R, side="left") + lo
            j = min(j, i + K)
            win_edge0[k].append(i)
            win_base[k].append(int(base))
            i = j
        win_edge0[k].append(hi)
    nb = max(len(wb) for wb in win_base)

    nslots = nb * K
    idx16 = np.full((NCORES, nslots), PADCOL - BIAS, dtype=np.int16)
    rowl_a = np.full((NCORES, nslots), PADROW, dtype=np.float16)
    w0_a = np.zeros((NCORES, nslots), dtype=np.float16)
    w1_a = np.zeros((NCORES, nslots), dtype=np.float16)
    rt_a = np.zeros((NCORES, nslots), dtype=np.float16)
    gidx = np.zeros((NCORES, nslots // KT // 128 * R), dtype=np.int64)

    for k in range(NCORES):
        e0 = np.asarray(win_edge0[k][:-1], dtype=np.int64)
        e1 = np.asarray(win_edge0[k][1:], dtype=np.int64)
        bases = np.asarray(win_base[k], dtype=np.int64)
        nwk = len(bases)
        cnt = e1 - e0
        # slot for edge i of window b: b*K + (i - e0[b])
        wid = np.repeat(np.arange(nwk), cnt)
        ed = np.arange(int(e0[0]) if nwk else 0, int(e1[-1]) if nwk else 0)
        slot = wid * K + (ed - e0[wid])
        idx16[k, slot] = (col_s[ed] - BIAS).astype(np.int16)
        rowl_a[k, slot] = (rl_s[ed] - bases[wid]).astype(np.float16)
        w0_a[k, slot] = w0_s[ed]
        w1_a[k, slot] = w1_s[ed]
        rt_a[k, slot] = rt_s[ed]
        # output column -> global row (clamp pads inside range; they add 0)
        gb = np.minimum(bases[:, None] + np.arange(R)[None, :], RPC - 1)
        gidx[k, :nwk * R] = (gb + k * RPC).reshape(-1)
        gidx[k, nwk * R:] = k * RPC  # unused windows: psum cols are zero

    # ensure the LAST slot of every gather chunk has idx >= 0 (dma_gather
    # trims a trailing negative run); swap within the final tile if needed
    cbounds = _chunk_bounds(nb)
    for cidx in range(len(cbounds) - 1):
        b1 = cbounds[cidx + 1]
        end = b1 * K                 # one past chunk's last slot
        for k in range(NCORES):
            if idx16[k, end - 1] < 0:
                tile_lo = end - 128
                cand = np.nonzero(idx16[k, tile_lo:end - 1] >= 0)[0]
                assert cand.size > 0, "entire tile has negative idx"
                j = tile_lo + cand[-1]
                for arr in (idx16, rowl_a, w0_a, w1_a, rt_a):
                    arr[k, j], arr[k, end - 1] = arr[k, end - 1], arr[k, j]

    tt = nb * KT
    iota_np = np.tile(np.arange(R, dtype=np.float16), (128, 1))
    in_maps = []
    for k in range(NCORES):
        in_maps.append({
            "hpre": H_pre,
            # idx position q -> partition q%16, free q//16; replicate x8
            "idx": np.ascontiguousarray(
                np.tile(idx16[k].reshape(nslots // 16, 16).T, (8, 1))),
            "rowl": np.ascontiguousarray(rowl_a[k].reshape(tt, 128).T),
            "w0": np.ascontiguousarray(w0_a[k].reshape(tt, 128).T),
            "w1": np.ascontiguousarray(w1_a[k].reshape(tt, 128).T),
            "rt": np.ascontiguousarray(rt_a[k].reshape(tt, 128).T),
            "iota": iota_np,
        })
    return nb, in_maps, gidx


def _make_runner(nc):
    """Build and cache a jitted shard_map executor for the compiled program."""
    import jax
    from jax.sharding import Mesh, PartitionSpec, NamedSharding
    from jax.experimental.shard_map import shard_map
    from concourse import mybir
    from concourse.bass2jax import (_bass_exec_p, partition_id_tensor,
                                    install_neuronx_cc_hook)

    install_neuronx_cc_hook()
    partition_name = nc.partition_id_tensor.name if nc.partition_id_tensor else None
    in_names, out_names, out_avals = [], [], []
    for alloc in nc.m.functions[0].allocations:
        if not isinstance(alloc, mybir.MemoryLocationSet):
            continue
        name = alloc.memorylocations[0].name
        if alloc.kind == "ExternalInput":
            if name != partition_name:
                in_names.append(name)
        elif alloc.kind == "ExternalOutput":
            out_names.append(name)
            out_avals.append(jax.core.ShapedArray(
                tuple(alloc.tensor_shape), mybir.dt.np(alloc.dtype)))
    n_params = len(in_names)
    all_in = in_names + out_names + ([partition_name] if partition_name else [])

    def _body(*args):
        operands = list(args)
        if partition_name is not None:
            operands.append(partition_id_tensor())
        return tuple(_bass_exec_p.bind(
            *operands, out_avals=tuple(out_avals), in_names=tuple(all_in),
            out_names=tuple(out_names), lowering_input_output_aliases=(),
            sim_require_finite=True, sim_require_nnan=True, nc=nc))

    devices = jax.devices()[:NCORES]
    mesh = Mesh(np.asarray(devices), ("core",))
    spec = PartitionSpec("core")
    f = jax.jit(shard_map(_body, mesh=mesh,
                          in_specs=(spec,) * (n_params + len(out_names)),
                          out_specs=(spec,), check_rep=False))
    sharding = NamedSharding(mesh, spec)
    zeros = [np.zeros((av.shape[0] * NCORES,) + av.shape[1:], av.dtype)
             for av in out_avals]
    return {"f": f, "in_names": in_names, "out_names": out_names,
            "sharding": sharding, "zeros": zeros}


def kernel(H_, edge_index, edge_values, weight, num_nodes):
    import jax

    nb, in_maps, gidx = _prepare(H_, edge_index, edge_values, weight)
    if nb not in _prog_cache:
        nc = _build_program(nb, **PROG_KW)
        _prog_cache[nb] = _make_runner(nc)
    rn = _prog_cache[nb]

    args = []
    for name in rn["in_names"]:
        glob = np.concatenate([m[name] for m in in_maps], axis=0)
        args.append(jax.device_put(glob, rn["sharding"]))
    for z in rn["zeros"]:
        args.append(jax.device_put(z, rn["sharding"]))
    outs = rn["f"](*args)
    res = np.asarray(outs[rn["out_names"].index("out_local")])  # [8*128, nb*R]

    out = np.zeros((C, N, D), dtype=np.float32)
    for k in range(NCORES):
        ol = res[k * 128:(k + 1) * 128]          # [128, nb*R]
        np.add.at(out[0], gidx[k], ol[0:D].T)
        np.add.at(out[1], gidx[k], ol[D:2 * D].T)
    return out
